# revision 2
# baseline (speedup 1.0000x reference)
"""GroupedQueryAttention Trainium2 kernel (8 NeuronCores).

Problem: B=4, S=N=2048, d_model=2048, G=16 heads, d_head=128,
RoPE (rotary_dim=512) applied to query only, key-position mask,
out = (softmax(mask(QK^T/sqrt(dh))) @ V) @ Wo^T.

Sharding: mesh = 4 batches x 2 head-halves. core_id = b*2 + h.
Each core: projections for its batch/head-half, attention for its 8 heads,
pair AllGather of context^T, O-projection of its 1024 output columns.

Everything is computed in fp32r (TF32) matmuls with fp32 accumulation.
Layouts are chosen so no on-device transposes are needed:
  - activations are fed transposed from the host (queryT/keyT/valueT),
  - Q^T/K^T (d_head x tokens) come straight out of the projection matmuls,
  - scores are computed transposed (keys on partitions), so the key mask
    folds into the exp's per-partition bias and P@V needs V in natural
    layout (also produced directly),
  - softmax denominator = ones-vector matmul accumulated alongside P@V.
"""
import sys
import numpy as np

sys.path.insert(0, "/opt/trn_rl_repo")

from contextlib import ExitStack

import concourse.bass as bass
import concourse.tile as tile
from concourse import bacc, mybir
from concourse.bass_utils import run_bass_kernel_spmd

FP32 = mybir.dt.float32
FP32R = mybir.dt.float32r

B = 4
S = 2048          # queries per batch
N = 2048          # keys per batch
D = 2048          # d_model
G = 16            # heads
DH = 128          # head dim
RD = 512          # rotary dim
TP = 2            # head-half split
CL = D // TP      # local channels (1024)
GL = G // TP      # local heads (8)
OC = D // TP      # output cols per core (1024)
SCALE = 1.0 / float(np.sqrt(DH))
MASK_BIAS = -30000.0

KT = D // 128     # contraction k-tiles (16)
SC = S // 512     # query chunks (4)
NT = N // 128     # key tiles (16)
ST = S // 128     # query 128-tiles (16)
CT = D // 128     # context c-tiles (16)


def _build_program():
    nc = bacc.Bacc("TRN2", target_bir_lowering=False, debug=False, num_devices=8)

    # ---- external I/O (per-core contents differ; same shapes) ----
    xq = nc.dram_tensor("xq", [D, S], FP32, kind="ExternalInput").ap()    # query^T
    xk = nc.dram_tensor("xk", [D, N], FP32, kind="ExternalInput").ap()    # key^T
    xv = nc.dram_tensor("xv", [D, N], FP32, kind="ExternalInput").ap()    # value^T
    wq = nc.dram_tensor("wq", [D, CL], FP32, kind="ExternalInput").ap()   # Wq[hs,:]^T
    wk = nc.dram_tensor("wk", [D, CL], FP32, kind="ExternalInput").ap()
    wv = nc.dram_tensor("wv", [D, CL], FP32, kind="ExternalInput").ap()
    wo = nc.dram_tensor("wo", [D, OC], FP32, kind="ExternalInput").ap()   # Wo^T[:, ocs]
    cosT = nc.dram_tensor("cosT", [RD, S], FP32, kind="ExternalInput").ap()
    sinT = nc.dram_tensor("sinT", [RD, S], FP32, kind="ExternalInput").ap()  # signed
    biasm = nc.dram_tensor("biasm", [128, NT], FP32, kind="ExternalInput").ap()
    ones_c = nc.dram_tensor("ones_c", [128, 1], FP32, kind="ExternalInput").ap()
    ones_r = nc.dram_tensor("ones_r", [1, 128], FP32, kind="ExternalInput").ap()
    out = nc.dram_tensor("out", [S, OC], FP32, kind="ExternalOutput").ap()

    # ---- DRAM scratch ----
    qT_d = nc.dram_tensor("qT_d", [CL, S], FP32).ap()        # rope'd Q^T spill
    v_d = nc.dram_tensor("v_d", [N, CL], FP32).ap()          # V natural spill
    ct_local = nc.dram_tensor("ct_local", [CL, S], FP32).ap()
    ct_gath = nc.dram_tensor("ct_gath", [D, S], FP32).ap()

    xq_r = xq.rearrange("(kt p) s -> kt p s", p=128).bitcast(FP32R)
    xk_r = xk.rearrange("(kt p) s -> kt p s", p=128).bitcast(FP32R)
    xv_r = xv.rearrange("(kt p) s -> kt p s", p=128).bitcast(FP32R)

    with tile.TileContext(nc) as tc:
        with ExitStack() as top:
            consts = top.enter_context(tc.tile_pool(name="consts", bufs=1))
            bias_t = consts.tile([128, NT], FP32)
            ones_ct = consts.tile([128, 1], FP32R)
            ones_rt = consts.tile([1, 128], FP32R)
            nc.sync.dma_start(out=bias_t, in_=biasm)
            nc.sync.dma_start(out=ones_ct, in_=ones_c.bitcast(FP32R))
            nc.sync.dma_start(out=ones_rt, in_=ones_r.bitcast(FP32R))

            # ---------- Phase Q: Q-projection + RoPE -> qT_d ----------
            with ExitStack() as ph:
                wpool = ph.enter_context(tc.tile_pool(name="wqpool", bufs=1))
                rpool = ph.enter_context(tc.tile_pool(name="ropepool", bufs=1))
                xpool = ph.enter_context(tc.tile_pool(name="xqpool", bufs=3))
                tpool = ph.enter_context(tc.tile_pool(name="qtmppool", bufs=2))
                rsc = ph.enter_context(tc.tile_pool(name="ropescratch", bufs=1))
                opool = ph.enter_context(tc.tile_pool(name="qoutpool", bufs=2))
                pps = ph.enter_context(tc.tile_pool(name="qps", bufs=1, space="PSUM"))

                wq_t = wpool.tile([128, KT, CL], FP32R)
                nc.sync.dma_start(
                    out=wq_t, in_=wq.rearrange("(kt p) c -> p kt c", p=128).bitcast(FP32R)
                )
                cos_t = rpool.tile([128, 4, S], FP32)
                sin_t = rpool.tile([128, 4, S], FP32)
                nc.sync.dma_start(out=cos_t, in_=cosT.rearrange("(gt p) s -> p gt s", p=128))
                nc.sync.dma_start(out=sin_t, in_=sinT.rearrange("(gt p) s -> p gt s", p=128))

                for sc in range(SC):
                    psums = []
                    for g in range(GL):
                        p = pps.tile([128, 512], FP32, name=f"qp{g}", tag=f"qp{g}")
                        psums.append(p)
                    for kt in range(KT):
                        x_t = xpool.tile([128, 512], FP32R, name="xq_t", tag="x")
                        nc.sync.dma_start(out=x_t, in_=xq_r[kt][:, sc * 512:(sc + 1) * 512])
                        for g in range(GL):
                            nc.tensor.matmul(
                                out=psums[g],
                                lhsT=wq_t[:, kt, g * 128:(g + 1) * 128],
                                rhs=x_t,
                                start=(kt == 0),
                                stop=(kt == KT - 1),
                            )
                    # copy all 8 head-slices to SBUF (fp32); rope rewrites g<4
                    tmp = tpool.tile([128, GL, 512], FP32, name="qtmp", tag="qtmp")
                    for g in range(GL):
                        nc.vector.tensor_copy(out=tmp[:, g, :], in_=psums[g])
                    qr = opool.tile([128, GL, 512], FP32R, name="qrope", tag="qrope")
                    sA = rsc.tile([128, 512], FP32, name="ropeA", tag="ropeA")
                    sB = rsc.tile([128, 512], FP32, name="ropeB", tag="ropeB")
                    for g in range(GL):
                        if g < 4:
                            ssl = slice(sc * 512, (sc + 1) * 512)
                            nc.vector.tensor_mul(out=sA, in0=tmp[:, g, :], in1=cos_t[:, g, ssl])
                            nc.vector.tensor_mul(out=sB, in0=tmp[:, g ^ 2, :], in1=sin_t[:, g, ssl])
                            nc.vector.tensor_add(out=qr[:, g, :], in0=sA, in1=sB)
                        else:
                            nc.vector.tensor_copy(out=qr[:, g, :], in_=tmp[:, g, :])
                    nc.sync.dma_start(
                        out=qT_d.rearrange("(g p) s -> p g s", p=128)[:, :, sc * 512:(sc + 1) * 512],
                        in_=qr.bitcast(FP32),
                    )

            # K^T lives from phase K until end of attention
            kpool = top.enter_context(tc.tile_pool(name="kpool", bufs=1))
            k_sb = kpool.tile([128, GL, NT, 128], FP32R)

            # ---------- Phase K: K-projection -> k_sb (resident) ----------
            with ExitStack() as ph:
                wpool = ph.enter_context(tc.tile_pool(name="wkpool", bufs=1))
                xpool = ph.enter_context(tc.tile_pool(name="xkpool", bufs=4))
                pps = ph.enter_context(tc.tile_pool(name="kps", bufs=1, space="PSUM"))

                wk_t = wpool.tile([128, KT, CL], FP32R)
                nc.sync.dma_start(
                    out=wk_t, in_=wk.rearrange("(kt p) c -> p kt c", p=128).bitcast(FP32R)
                )
                for nch in range(N // 512):
                    psums = []
                    for g in range(GL):
                        p = pps.tile([128, 512], FP32, name=f"kp{g}", tag=f"kp{g}")
                        psums.append(p)
                    for kt in range(KT):
                        x_t = xpool.tile([128, 512], FP32R, name="xk_t", tag="x")
                        nc.sync.dma_start(out=x_t, in_=xk_r[kt][:, nch * 512:(nch + 1) * 512])
                        for g in range(GL):
                            nc.tensor.matmul(
                                out=psums[g],
                                lhsT=wk_t[:, kt, g * 128:(g + 1) * 128],
                                rhs=x_t,
                                start=(kt == 0),
                                stop=(kt == KT - 1),
                            )
                    for g in range(GL):
                        nc.vector.tensor_copy(
                            out=k_sb[:, g, nch * 4:(nch + 1) * 4, :],
                            in_=psums[g].rearrange("p (a b) -> p a b", b=128),
                        )

            # ---------- Phase V: V-projection -> v_d (natural layout) ----------
            with ExitStack() as ph:
                wpool = ph.enter_context(tc.tile_pool(name="wvpool", bufs=1))
                xpool = ph.enter_context(tc.tile_pool(name="xvpool", bufs=2))
                vopool = ph.enter_context(tc.tile_pool(name="vout", bufs=3))
                pps = ph.enter_context(tc.tile_pool(name="vps", bufs=2, space="PSUM"))

                wv_t = wpool.tile([128, KT, CL], FP32R)
                nc.sync.dma_start(
                    out=wv_t, in_=wv.rearrange("(kt p) c -> p kt c", p=128).bitcast(FP32R)
                )
                for nt4 in range(N // 512):
                    # lhsT tiles: xv columns for 4 n-tiles at once
                    x_t = xpool.tile([128, KT, 512], FP32R, name="xv_t", tag="x")
                    nc.sync.dma_start(
                        out=x_t,
                        in_=xv_r.rearrange("kt p s -> p kt s")[:, :, nt4 * 512:(nt4 + 1) * 512],
                    )
                    for j in range(4):
                        psums = []
                        for cc in range(2):
                            p = pps.tile([128, 512], FP32, name=f"vp{cc}", tag=f"vp{cc}")
                            psums.append(p)
                        for kt in range(KT):
                            for cc in range(2):
                                nc.tensor.matmul(
                                    out=psums[cc],
                                    lhsT=x_t[:, kt, j * 128:(j + 1) * 128],
                                    rhs=wv_t[:, kt, cc * 512:(cc + 1) * 512],
                                    start=(kt == 0),
                                    stop=(kt == KT - 1),
                                )
                        v_o = vopool.tile([128, CL], FP32, name="v_o", tag="v_o")
                        for cc in range(2):
                            nc.vector.tensor_copy(
                                out=v_o[:, cc * 512:(cc + 1) * 512], in_=psums[cc]
                            )
                        nt = nt4 * 4 + j
                        nc.sync.dma_start(out=v_d[nt * 128:(nt + 1) * 128, :], in_=v_o)

            # ---------- Phase A: attention per head -> ct_local ----------
            with ExitStack() as ph:
                spool = ph.enter_context(tc.tile_pool(name="slabs", bufs=2))
                epool = ph.enter_context(tc.tile_pool(name="epool", bufs=4))
                cpool = ph.enter_context(tc.tile_pool(name="cpool", bufs=4))
                rpool = ph.enter_context(tc.tile_pool(name="rpool", bufs=4))
                sps = ph.enter_context(tc.tile_pool(name="sps", bufs=2, space="PSUM"))
                ups = ph.enter_context(tc.tile_pool(name="ups", bufs=2, space="PSUM"))
                dps = ph.enter_context(tc.tile_pool(name="dps", bufs=2, space="PSUM"))
                bps = ph.enter_context(tc.tile_pool(name="bps", bufs=2, space="PSUM"))

                v_dr = v_d.rearrange("(nt p) c -> p nt c", p=128).bitcast(FP32R)
                q_dr = qT_d.rearrange("(g p) s -> g p s", p=128).bitcast(FP32R)
                for g in range(GL):
                    v_sb = spool.tile([128, NT, 128], FP32R, name="v_sb", tag="v_sb")
                    nc.sync.dma_start(out=v_sb, in_=v_dr[:, :, g * 128:(g + 1) * 128])
                    q_sb = spool.tile([128, SC, 512], FP32R, name="q_sb", tag="q_sb")
                    nc.sync.dma_start(
                        out=q_sb, in_=q_dr[g].rearrange("p (a b) -> p a b", b=512)
                    )
                    for sc in range(SC):
                        u_ps = ups.tile([128, 512], FP32, name="u_ps", tag="u")
                        d_ps = dps.tile([1, 512], FP32, name="d_ps", tag="d")
                        for nt in range(NT):
                            s_ps = sps.tile([128, 512], FP32, name="s_ps", tag="s")
                            nc.tensor.matmul(
                                out=s_ps, lhsT=k_sb[:, g, nt, :], rhs=q_sb[:, sc, :],
                                start=True, stop=True,
                            )
                            e_t = epool.tile([128, 512], FP32R, name="e_t", tag="e")
                            nc.scalar.activation(
                                out=e_t, in_=s_ps,
                                func=mybir.ActivationFunctionType.Exp,
                                bias=bias_t[:, nt:nt + 1], scale=SCALE,
                            )
                            nc.tensor.matmul(
                                out=d_ps, lhsT=ones_ct, rhs=e_t,
                                start=(nt == 0), stop=(nt == NT - 1),
                            )
                            nc.tensor.matmul(
                                out=u_ps, lhsT=v_sb[:, nt, :], rhs=e_t,
                                start=(nt == 0), stop=(nt == NT - 1),
                            )
                        r_t = rpool.tile([1, 512], FP32R, name="r_t", tag="r")
                        with nc.allow_low_precision(reason="tf32 softmax scale"):
                            nc.vector.reciprocal(out=r_t, in_=d_ps)
                        b_ps = bps.tile([128, 512], FP32, name="b_ps", tag="b")
                        nc.tensor.matmul(out=b_ps, lhsT=ones_rt, rhs=r_t,
                                         start=True, stop=True)
                        b_sb = rpool.tile([128, 512], FP32, name="b_sb", tag="bsb")
                        nc.vector.tensor_copy(out=b_sb, in_=b_ps)
                        c_t = cpool.tile([128, 512], FP32, name="c_t", tag="c")
                        nc.vector.tensor_mul(out=c_t, in0=u_ps, in1=b_sb)
                        nc.sync.dma_start(
                            out=ct_local[g * 128:(g + 1) * 128, sc * 512:(sc + 1) * 512],
                            in_=c_t,
                        )

            # ---------- AllGather context^T within batch pairs ----------
            nc.gpsimd.collective_compute(
                "AllGather",
                mybir.AluOpType.bypass,
                replica_groups=[[0, 1], [2, 3], [4, 5], [6, 7]],
                ins=[ct_local],
                outs=[ct_gath],
            )

            # ---------- Phase O: out = C @ Wo^T (half columns) ----------
            with ExitStack() as ph:
                wpool = ph.enter_context(tc.tile_pool(name="wopool", bufs=1))
                cpool = ph.enter_context(tc.tile_pool(name="ctpool", bufs=3))
                oopool = ph.enter_context(tc.tile_pool(name="oout", bufs=3))
                pps = ph.enter_context(tc.tile_pool(name="ops", bufs=2, space="PSUM"))

                wo_t = wpool.tile([128, CT, OC], FP32R)
                nc.sync.dma_start(
                    out=wo_t, in_=wo.rearrange("(ct p) c -> p ct c", p=128).bitcast(FP32R)
                )
                ct_r = ct_gath.rearrange("(ct p) s -> p ct s", p=128).bitcast(FP32R)
                for st in range(ST):
                    c_sb = cpool.tile([128, CT, 128], FP32R, name="c_sb", tag="c_sb")
                    nc.sync.dma_start(
                        out=c_sb, in_=ct_r[:, :, st * 128:(st + 1) * 128]
                    )
                    psums = []
                    for cc in range(2):
                        p = pps.tile([128, 512], FP32, name=f"op{cc}", tag=f"op{cc}")
                        psums.append(p)
                    for ct in range(CT):
                        for cc in range(2):
                            nc.tensor.matmul(
                                out=psums[cc],
                                lhsT=c_sb[:, ct, :],
                                rhs=wo_t[:, ct, cc * 512:(cc + 1) * 512],
                                start=(ct == 0),
                                stop=(ct == CT - 1),
                            )
                    o_sb = oopool.tile([128, OC], FP32, name="o_sb", tag="o_sb")
                    for cc in range(2):
                        nc.vector.tensor_copy(
                            out=o_sb[:, cc * 512:(cc + 1) * 512], in_=psums[cc]
                        )
                    nc.sync.dma_start(out=out[st * 128:(st + 1) * 128, :], in_=o_sb)

    nc.compile()
    return nc


_NC_CACHE = {}


def _get_program():
    if "nc" not in _NC_CACHE:
        _NC_CACHE["nc"] = _build_program()
    return _NC_CACHE["nc"]


def kernel(query, key, value, mask, position_ids, Wq, Wk, Wv, Wo, **kw):
    query = np.asarray(query, dtype=np.float32)
    key = np.asarray(key, dtype=np.float32)
    value = np.asarray(value, dtype=np.float32)
    mask = np.asarray(mask)
    position_ids = np.asarray(position_ids)
    Wq = np.asarray(Wq, dtype=np.float32)
    Wk = np.asarray(Wk, dtype=np.float32)
    Wv = np.asarray(Wv, dtype=np.float32)
    Wo = np.asarray(Wo, dtype=np.float32)

    # rope tables from actual position_ids (applied to query only)
    pos = position_ids.astype(np.float64)  # (S,)
    freq = np.arange(0, RD, 2, dtype=np.float64)
    inv_freq = 1.0 / (10000.0 ** (freq / RD))  # (RD/2,)
    pe = pos[:, None] * inv_freq[None, :]      # (S, RD/2)
    cos_half = np.cos(pe).astype(np.float32)   # (S, 256)
    sin_half = np.sin(pe).astype(np.float32)
    cosT_full = np.tile(cos_half, (1, 2)).T.copy()      # (512, S)
    sin_full = np.tile(sin_half, (1, 2)).T               # (512, S)
    sinT_signed = sin_full.copy()
    sinT_signed[: RD // 2] *= -1.0                       # partner sign
    cosT_id = np.ones((RD, S), np.float32)
    sinT_id = np.zeros((RD, S), np.float32)

    ones_c = np.ones((128, 1), np.float32)
    ones_r = np.ones((1, 128), np.float32)

    in_maps = []
    for core in range(8):
        b, h = core // 2, core % 2
        hs = slice(h * CL, (h + 1) * CL)
        biasm = ((mask[b].astype(np.float32) - 1.0) * -MASK_BIAS * -1.0)
        # mask==1 -> 0 ; mask==0 -> MASK_BIAS
        biasm = np.where(mask[b] == 0, np.float32(MASK_BIAS), np.float32(0.0))
        in_maps.append({
            "xq": np.ascontiguousarray(query[b].T),
            "xk": np.ascontiguousarray(key[b].T),
            "xv": np.ascontiguousarray(value[b].T),
            "wq": np.ascontiguousarray(Wq[hs, :].T),
            "wk": np.ascontiguousarray(Wk[hs, :].T),
            "wv": np.ascontiguousarray(Wv[hs, :].T),
            "wo": np.ascontiguousarray(Wo.T[:, hs]),
            "cosT": cosT_full if h == 0 else cosT_id,
            "sinT": sinT_signed if h == 0 else sinT_id,
            "biasm": np.ascontiguousarray(biasm.reshape(NT, 128).T),
            "ones_c": ones_c,
            "ones_r": ones_r,
        })

    nc = _get_program()
    res = run_bass_kernel_spmd(nc, in_maps, core_ids=list(range(8)))
    _NC_CACHE["last_res"] = res

    out = np.empty((B, S, D), np.float32)
    for core in range(8):
        b, h = core // 2, core % 2
        out[b][:, h * OC:(h + 1) * OC] = res.results[core]["out"]
    return out



# revision 5
# speedup vs baseline: 1.5357x; 1.5357x over previous
"""GroupedQueryAttention Trainium2 kernel (8 NeuronCores).

Problem: B=4, S=N=2048, d_model=2048, G=16 heads, d_head=128,
RoPE (rotary_dim=512) applied to query only, key-position mask,
out = (softmax(mask(QK^T/sqrt(dh))) @ V) @ Wo^T.

Sharding: mesh = 4 batches x 2 head-halves. core_id = b*2 + h.
Each core: projections for its batch/head-half, attention for its 8 heads,
chunked (per-512-query) AllGather of context^T in fp16 overlapped with
attention on the next chunk, O-projection per chunk interleaved on PE.

Projections run in fp32r (TF32) matmuls; attention (Q/K/V post-projection,
exp weights, context) and the O-projection run in fp16 — all PSUM
accumulation stays fp32.

Softmax denominator: e-tiles are accumulated (fp16, DVE 2x) into esum,
then ONE matmul with an all-ones [128,128] lhsT produces the denominator
already broadcast across all 128 partitions; a fast approximate
reciprocal and one fused multiply normalize the context.
"""
import sys
import numpy as np

sys.path.insert(0, "/opt/trn_rl_repo")

from contextlib import ExitStack

import concourse.bass as bass
import concourse.tile as tile
from concourse import bacc, mybir
from concourse.bass_utils import run_bass_kernel_spmd

FP32 = mybir.dt.float32
FP32R = mybir.dt.float32r
FP16 = mybir.dt.float16

B = 4
S = 2048          # queries per batch
N = 2048          # keys per batch
D = 2048          # d_model
G = 16            # heads
DH = 128          # head dim
RD = 512          # rotary dim
TP = 2            # head-half split
CL = D // TP      # local channels (1024)
GL = G // TP      # local heads (8)
OC = D // TP      # output cols per core (1024)
SCALE = 1.0 / float(np.sqrt(DH))
MASK_BIAS = -30000.0

KT = D // 128     # contraction k-tiles (16)
SC = S // 512     # query chunks (4)
NT = N // 128     # key tiles (16)
CT = D // 128     # context c-tiles (16)


def _build_program():
    nc = bacc.Bacc("TRN2", target_bir_lowering=False, debug=False, num_devices=8)

    # ---- external I/O (per-core contents differ; same shapes) ----
    xq = nc.dram_tensor("xq", [D, S], FP32, kind="ExternalInput").ap()    # query^T
    xk = nc.dram_tensor("xk", [D, N], FP32, kind="ExternalInput").ap()    # key^T
    xv = nc.dram_tensor("xv", [D, N], FP32, kind="ExternalInput").ap()    # value^T
    wq = nc.dram_tensor("wq", [D, CL], FP32, kind="ExternalInput").ap()   # Wq[hs,:]^T
    wk = nc.dram_tensor("wk", [D, CL], FP32, kind="ExternalInput").ap()
    wv = nc.dram_tensor("wv", [D, CL], FP32, kind="ExternalInput").ap()
    wo = nc.dram_tensor("wo", [D, OC], FP16, kind="ExternalInput").ap()   # Wo^T[:, ocs] fp16
    cosT = nc.dram_tensor("cosT", [RD // 2, S], FP16, kind="ExternalInput").ap()
    sinT = nc.dram_tensor("sinT", [RD // 2, S], FP16, kind="ExternalInput").ap()
    biasm = nc.dram_tensor("biasm", [128, NT], FP32, kind="ExternalInput").ap()
    ones_in = nc.dram_tensor("ones_in", [128, 128], FP16, kind="ExternalInput").ap()
    out = nc.dram_tensor("out", [S, OC], FP32, kind="ExternalOutput").ap()

    # ---- DRAM scratch: per-query-chunk context halves + gathered ----
    ct_d = [nc.dram_tensor(f"ct{i}", [CL, 512], FP16).ap() for i in range(SC)]
    ctg_d = [nc.dram_tensor(f"ctg{i}", [D, 512], FP16).ap() for i in range(SC)]

    xq_r = xq.rearrange("(kt p) s -> kt p s", p=128).bitcast(FP32R)
    xk_r = xk.rearrange("(kt p) s -> kt p s", p=128).bitcast(FP32R)
    xv_r = xv.rearrange("(kt p) s -> kt p s", p=128).bitcast(FP32R)
    wq_r = wq.rearrange("(kt p) c -> p kt c", p=128).bitcast(FP32R)
    wk_r = wk.rearrange("(kt p) c -> p kt c", p=128).bitcast(FP32R)
    wv_r = wv.rearrange("(kt p) c -> p kt c", p=128).bitcast(FP32R)
    wo_r = wo.rearrange("(ct p) c -> p ct c", p=128)
    cos_r = cosT.rearrange("(gt p) s -> p gt s", p=128)
    sin_r = sinT.rearrange("(gt p) s -> p gt s", p=128)

    with tile.TileContext(nc) as tc:
        with ExitStack() as top:
            consts = top.enter_context(tc.tile_pool(name="consts", bufs=1))
            bias_t = consts.tile([128, NT], FP32)
            ones_sq = consts.tile([128, 128], FP16)

            # persistent activation tiles (fp16): Q (post-rope), K^T, V
            qkpool = top.enter_context(tc.tile_pool(name="qkpool", bufs=1))
            q_sb = qkpool.tile([128, GL, SC, 512], FP16)   # [dh, g, sc, s]
            k_sb = qkpool.tile([128, GL, NT, 128], FP16)   # [dh, g, nt, n]
            v_sb = qkpool.tile([128, NT, CL], FP16)        # [n, nt, c]

            nc.gpsimd.dma_start(out=bias_t, in_=biasm)
            nc.gpsimd.dma_start(out=ones_sq, in_=ones_in)

            # ---------- Phase Q: Q-projection + RoPE -> q_sb ----------
            with ExitStack() as ph:
                wpool = ph.enter_context(tc.tile_pool(name="wqpool", bufs=1))
                rpool = ph.enter_context(tc.tile_pool(name="ropepool", bufs=1))
                xpool = ph.enter_context(tc.tile_pool(name="xqpool", bufs=3))
                tpool = ph.enter_context(tc.tile_pool(name="qtmppool", bufs=2))
                rsc = ph.enter_context(tc.tile_pool(name="ropescratch", bufs=2))
                pps = ph.enter_context(tc.tile_pool(name="qps", bufs=1, space="PSUM"))

                wq_t = wpool.tile([128, KT, CL], FP32R)
                for kt in range(KT):
                    nc.gpsimd.dma_start(out=wq_t[:, kt, :], in_=wq_r[:, kt, :])
                cos_t = rpool.tile([128, 2, S], FP16)
                sin_t = rpool.tile([128, 2, S], FP16)
                nc.gpsimd.dma_start(out=cos_t, in_=cos_r)
                nc.gpsimd.dma_start(out=sin_t, in_=sin_r)

                for sc in range(SC):
                    psums = []
                    for g in range(GL):
                        p = pps.tile([128, 512], FP32, name=f"qp{g}", tag=f"qp{g}")
                        psums.append(p)
                    for kt in range(KT):
                        x_t = xpool.tile([128, 512], FP32R, name="xq_t", tag="x")
                        nc.sync.dma_start(out=x_t, in_=xq_r[kt][:, sc * 512:(sc + 1) * 512])
                        for g in range(GL):
                            nc.tensor.matmul(
                                out=psums[g],
                                lhsT=wq_t[:, kt, g * 128:(g + 1) * 128],
                                rhs=x_t,
                                start=(kt == 0),
                                stop=(kt == KT - 1),
                            )
                    # rope heads (g<4) need fp32 copies; rest cast straight to fp16
                    tmp = tpool.tile([128, 4, 512], FP16, name="qtmp", tag="qtmp")
                    for g in range(4):
                        nc.scalar.copy(out=tmp[:, g, :], in_=psums[g])
                    for g in range(4, GL):
                        nc.scalar.copy(out=q_sb[:, g, sc, :], in_=psums[g])
                    ssl = slice(sc * 512, (sc + 1) * 512)
                    for g in range(4):
                        sA = rsc.tile([128, 512], FP16, name="ropeA", tag="ropeA")
                        sB = rsc.tile([128, 512], FP16, name="ropeB", tag="ropeB")
                        nc.vector.tensor_mul(out=sA, in0=tmp[:, g, :], in1=cos_t[:, g % 2, ssl])
                        nc.vector.tensor_mul(out=sB, in0=tmp[:, g ^ 2, :], in1=sin_t[:, g % 2, ssl])
                        if g < 2:
                            nc.vector.tensor_sub(out=q_sb[:, g, sc, :], in0=sA, in1=sB)
                        else:
                            nc.vector.tensor_add(out=q_sb[:, g, sc, :], in0=sA, in1=sB)

            # ---------- Phase K: K-projection -> k_sb ----------
            with ExitStack() as ph:
                wpool = ph.enter_context(tc.tile_pool(name="wkpool", bufs=1))
                xpool = ph.enter_context(tc.tile_pool(name="xkpool", bufs=3))
                pps = ph.enter_context(tc.tile_pool(name="kps", bufs=1, space="PSUM"))

                wk_t = wpool.tile([128, KT, CL], FP32R)
                for kt in range(KT):
                    nc.gpsimd.dma_start(out=wk_t[:, kt, :], in_=wk_r[:, kt, :])
                for nch in range(N // 512):
                    psums = []
                    for g in range(GL):
                        p = pps.tile([128, 512], FP32, name=f"kp{g}", tag=f"kp{g}")
                        psums.append(p)
                    for kt in range(KT):
                        x_t = xpool.tile([128, 512], FP32R, name="xk_t", tag="x")
                        nc.sync.dma_start(out=x_t, in_=xk_r[kt][:, nch * 512:(nch + 1) * 512])
                        for g in range(GL):
                            nc.tensor.matmul(
                                out=psums[g],
                                lhsT=wk_t[:, kt, g * 128:(g + 1) * 128],
                                rhs=x_t,
                                start=(kt == 0),
                                stop=(kt == KT - 1),
                            )
                    for g in range(GL):
                        nc.scalar.copy(
                            out=k_sb[:, g, nch * 4:(nch + 1) * 4, :],
                            in_=psums[g].rearrange("p (a b) -> p a b", b=128),
                        )

            # ---------- Phase V: V-projection -> v_sb (natural layout) ----------
            with ExitStack() as ph:
                wpool = ph.enter_context(tc.tile_pool(name="wvpool", bufs=1))
                xpool = ph.enter_context(tc.tile_pool(name="xvpool", bufs=3))
                pps = ph.enter_context(tc.tile_pool(name="vps", bufs=1, space="PSUM"))

                wv_t = wpool.tile([128, KT, CL], FP32R)
                for kt in range(KT):
                    nc.gpsimd.dma_start(out=wv_t[:, kt, :], in_=wv_r[:, kt, :])
                for nt4 in range(N // 512):
                    psums = []
                    for j in range(4):
                        for cc in range(2):
                            p = pps.tile(
                                [128, 512], FP32, name=f"vp{j}{cc}", tag=f"vp{j}{cc}"
                            )
                            psums.append(p)
                    for kt in range(KT):
                        x_t = xpool.tile([128, 512], FP32R, name="xv_t", tag="x")
                        nc.sync.dma_start(out=x_t, in_=xv_r[kt][:, nt4 * 512:(nt4 + 1) * 512])
                        for j in range(4):
                            for cc in range(2):
                                nc.tensor.matmul(
                                    out=psums[j * 2 + cc],
                                    lhsT=x_t[:, j * 128:(j + 1) * 128],
                                    rhs=wv_t[:, kt, cc * 512:(cc + 1) * 512],
                                    start=(kt == 0),
                                    stop=(kt == KT - 1),
                                )
                    for j in range(4):
                        for cc in range(2):
                            nc.scalar.copy(
                                out=v_sb[:, nt4 * 4 + j, cc * 512:(cc + 1) * 512],
                                in_=psums[j * 2 + cc],
                            )

            # ---------- Attention (per query chunk) + chunked gather + O ----------
            with ExitStack() as ph:
                wpool = ph.enter_context(tc.tile_pool(name="wopool", bufs=1))
                epool = ph.enter_context(tc.tile_pool(name="epool", bufs=4))
                espool = ph.enter_context(tc.tile_pool(name="espool", bufs=2))
                rpool = ph.enter_context(tc.tile_pool(name="rpool", bufs=2))
                cpool = ph.enter_context(tc.tile_pool(name="cpool", bufs=2))
                csbpool = ph.enter_context(tc.tile_pool(name="csbpool", bufs=2))
                osbpool = ph.enter_context(tc.tile_pool(name="osbpool", bufs=2))
                sps = ph.enter_context(tc.tile_pool(name="sps", bufs=2, space="PSUM"))
                ups = ph.enter_context(tc.tile_pool(name="ups", bufs=2, space="PSUM"))
                dps = ph.enter_context(tc.tile_pool(name="dps", bufs=2, space="PSUM"))
                ops = ph.enter_context(tc.tile_pool(name="ops", bufs=1, space="PSUM"))

                wo_t = wpool.tile([128, CT, OC], FP16)
                for ct in range(CT):
                    nc.gpsimd.dma_start(out=wo_t[:, ct, :], in_=wo_r[:, ct, :])

                def emit_attn(sc):
                    for g in range(GL):
                        u_ps = ups.tile([128, 512], FP32, name="u_ps", tag="u")
                        esum = espool.tile([128, 512], FP16, name="esum", tag="es")
                        for nt in range(NT):
                            s_ps = sps.tile([128, 512], FP32, name="s_ps", tag="s")
                            nc.tensor.matmul(
                                out=s_ps,
                                lhsT=k_sb[:, g, nt, :],
                                rhs=q_sb[:, g, sc, :],
                                start=True,
                                stop=True,
                            )
                            e_t = epool.tile([128, 512], FP16, name="e_t", tag="e")
                            nc.scalar.activation(
                                out=e_t, in_=s_ps,
                                func=mybir.ActivationFunctionType.Exp,
                                bias=bias_t[:, nt:nt + 1], scale=SCALE,
                            )
                            with nc.allow_low_precision(reason="fp16 esum accumulate"):
                                if nt == 0:
                                    nc.vector.tensor_copy(out=esum, in_=e_t)
                                else:
                                    nc.vector.tensor_add(out=esum, in0=esum, in1=e_t)
                            nc.tensor.matmul(
                                out=u_ps,
                                lhsT=v_sb[:, nt, g * 128:(g + 1) * 128],
                                rhs=e_t,
                                start=(nt == 0),
                                stop=(nt == NT - 1),
                            )
                        # denominator, broadcast across partitions in one matmul
                        d_ps = dps.tile([128, 512], FP32, name="d_ps", tag="d")
                        nc.tensor.matmul(
                            out=d_ps, lhsT=ones_sq, rhs=esum, start=True, stop=True
                        )
                        r_t = rpool.tile([128, 512], FP32, name="r_t", tag="r")
                        nc.vector.reciprocal_approx_fast(out=r_t, in_=d_ps)
                        c_t = cpool.tile([128, 512], FP16, name="c_t", tag="c")
                        nc.vector.tensor_mul(out=c_t, in0=u_ps, in1=r_t)
                        nc.sync.dma_start(
                            out=ct_d[sc][g * 128:(g + 1) * 128, :], in_=c_t
                        )
                    nc.gpsimd.collective_compute(
                        "AllGather",
                        mybir.AluOpType.bypass,
                        replica_groups=[[0, 1], [2, 3], [4, 5], [6, 7]],
                        ins=[ct_d[sc]],
                        outs=[ctg_d[sc]],
                    )

                def emit_o(j):
                    ctg_r = ctg_d[j].rearrange("(ct p) s -> p ct s", p=128)
                    for st4 in range(4):
                        st = j * 4 + st4
                        c_sb = csbpool.tile(
                            [128, CT, 128], FP16, name="c_sb", tag="c_sb"
                        )
                        nc.sync.dma_start(
                            out=c_sb, in_=ctg_r[:, :, st4 * 128:(st4 + 1) * 128]
                        )
                        psums = []
                        for cc in range(2):
                            p = ops.tile([128, 512], FP32, name=f"op{cc}", tag=f"op{cc}")
                            psums.append(p)
                        for ct in range(CT):
                            for cc in range(2):
                                nc.tensor.matmul(
                                    out=psums[cc],
                                    lhsT=c_sb[:, ct, :],
                                    rhs=wo_t[:, ct, cc * 512:(cc + 1) * 512],
                                    start=(ct == 0),
                                    stop=(ct == CT - 1),
                                )
                        o_sb = osbpool.tile([128, OC], FP32, name="o_sb", tag="o_sb")
                        for cc in range(2):
                            nc.vector.tensor_copy(
                                out=o_sb[:, cc * 512:(cc + 1) * 512], in_=psums[cc]
                            )
                        nc.sync.dma_start(out=out[st * 128:(st + 1) * 128, :], in_=o_sb)

                emit_attn(0)
                emit_attn(1)
                emit_o(0)
                emit_attn(2)
                emit_o(1)
                emit_attn(3)
                emit_o(2)
                emit_o(3)

    nc.compile()
    return nc


_NC_CACHE = {}


def _get_program():
    if "nc" not in _NC_CACHE:
        _NC_CACHE["nc"] = _build_program()
    return _NC_CACHE["nc"]


def kernel(query, key, value, mask, position_ids, Wq, Wk, Wv, Wo, **kw):
    query = np.asarray(query, dtype=np.float32)
    key = np.asarray(key, dtype=np.float32)
    value = np.asarray(value, dtype=np.float32)
    mask = np.asarray(mask)
    position_ids = np.asarray(position_ids)
    Wq = np.asarray(Wq, dtype=np.float32)
    Wk = np.asarray(Wk, dtype=np.float32)
    Wv = np.asarray(Wv, dtype=np.float32)
    Wo = np.asarray(Wo, dtype=np.float32)

    # rope tables from actual position_ids (applied to query only)
    pos = position_ids.astype(np.float64)  # (S,)
    freq = np.arange(0, RD, 2, dtype=np.float64)
    inv_freq = 1.0 / (10000.0 ** (freq / RD))  # (RD/2,)
    pe = pos[:, None] * inv_freq[None, :]      # (S, RD/2=256)
    cosT_half = np.ascontiguousarray(np.cos(pe).T.astype(np.float16))  # (256, S)
    sinT_half = np.ascontiguousarray(np.sin(pe).T.astype(np.float16))
    cosT_id = np.ones((RD // 2, S), np.float16)
    sinT_id = np.zeros((RD // 2, S), np.float16)

    ones_sq = np.ones((128, 128), np.float16)
    WoT = Wo.T.astype(np.float16)

    in_maps = []
    for core in range(8):
        b, h = core // 2, core % 2
        hs = slice(h * CL, (h + 1) * CL)
        biasm = np.where(mask[b] == 0, np.float32(MASK_BIAS), np.float32(0.0))
        in_maps.append({
            "xq": np.ascontiguousarray(query[b].T),
            "xk": np.ascontiguousarray(key[b].T),
            "xv": np.ascontiguousarray(value[b].T),
            "wq": np.ascontiguousarray(Wq[hs, :].T),
            "wk": np.ascontiguousarray(Wk[hs, :].T),
            "wv": np.ascontiguousarray(Wv[hs, :].T),
            "wo": np.ascontiguousarray(WoT[:, hs]),
            "cosT": cosT_half if h == 0 else cosT_id,
            "sinT": sinT_half if h == 0 else sinT_id,
            "biasm": np.ascontiguousarray(biasm.reshape(NT, 128).T),
            "ones_in": ones_sq,
        })

    nc = _get_program()
    res = run_bass_kernel_spmd(nc, in_maps, core_ids=list(range(8)))
    _NC_CACHE["last_res"] = res

    out = np.empty((B, S, D), np.float32)
    for core in range(8):
        b, h = core // 2, core % 2
        out[b][:, h * OC:(h + 1) * OC] = res.results[core]["out"]
    return out
